# revision 7
# baseline (speedup 1.0000x reference)
"""Ewald real-space potential kernel for Trainium2 (8 NeuronCores, SPMD).

Computes pot = 0.5 * sum_{i != j} q_i * erf(d_ij / sqrt(2)) / d_ij * C  over
all pairs of N=4096 atoms, C = 90.0474 / (2*pi).

Sharding: each core owns a 512-column i-slice of the 4096x4096 pair matrix.
On-chip layout per work tile is [128 j-partitions, 512 i-free]; j-blocks are
processed in super-groups of 6 (one [128, 3072] PSUM tensor, PE fills one
3-block half while ACT reads the other via bank-level dependency tracking):
  1. PE     : d2 = |r_i - r_j|^2 via a K=16 fp16 hi/lo-split matmul
              (fp16 products are exact in fp32 PSUM; |err| < 8e-3). The
              core's own j-window carries +0.01 on |r_j|^2 so the masked
              diagonal stays positive under that error; real pairs are
              unaffected (closest pair d2 = 0.0144).
  2. ACT    : dist = Sqrt(d2)            (PSUM -> SBUF, one op per super)
  3. DVE    : rinv = reciprocal_approx_fast(dist)   (~51 ULP)
  4. ACT    : erf_t = Erf(dist / sqrt(2))
  5. DVE    : zero erf_t on the self-interaction diagonal (mask multiply)
  6. DVE/GP : w = erf_t * rinv   (cast fp16, kept resident in SBUF)
  7. PE     : s[1,512] += q_j^T @ w  -- all 32 fp16 matmuls emitted after the
              elementwise loop so the PE FIFO never blocks d2 production.
  8. DVE/ACT: pot = C/2 * sum_i q_i * s_i
The j-axis is rotated per core so each core's own diagonal window lands in
j-blocks 28..31, letting one shared mask tensor serve all cores (SPMD).
Host sums the 8 per-core partial potentials.
"""

import numpy as np

P = 128
N = 4096
NCORES = 8
COLS = N // NCORES          # 512 i-columns per core
NB = N // P                 # 32 j-blocks
K = 16                      # split-matmul contraction rows
SIGMA = 1.0
NORM_CONST = 90.0474 / (2.0 * np.pi)
A = 1.0 / (SIGMA * np.sqrt(2.0))
DIAG_EPS = 0.01             # |r_j|^2 bias on the core's own j-window only
# super-groups: j-blocks per elementwise op; last group covers the remainder
SUPERS = [6, 6, 6, 6, 6, 2]
# which super-groups run the final w-multiply on gpsimd (rest on vector)
GPSIMD_SUPERS = frozenset({0, 3})

_CACHE = {}


def _split2(v32):
    h = v32.astype(np.float16)
    l = (v32 - h.astype(np.float32)).astype(np.float16)
    return h, l


def _build_core_inputs(q, r):
    """Per-core input arrays (keyed by dram tensor name)."""
    q = q.astype(np.float32)
    r = r.astype(np.float32)
    r2_64 = (r.astype(np.float64) ** 2).sum(1)

    # shared diagonal mask: Z[p, u] = 0 iff u == p + 384  (sliced per j-block)
    z = np.ones((P, 896), np.float32)
    z[np.arange(P), np.arange(P) + 384] = 0.0

    in_maps = []
    for c in range(NCORES):
        perm = (np.arange(N) + COLS * (c + 1)) % N  # rotated j order
        win = slice(COLS * c, COLS * (c + 1))       # this core's i-window

        rows_j, rows_i = [], []
        for d in range(3):
            cj = r[perm, d]
            ui = (-2.0 * r[win, d]).astype(np.float32)
            jh, jl = _split2(cj)
            ih, il = _split2(ui)
            rows_j += [jh, jh, jl, jl]
            rows_i += [ih, il, ih, il]
        r2j = r2_64[perm].copy()
        r2j[N - COLS:] += DIAG_EPS      # rotated slots 3584.. = own window
        r2j = r2j.astype(np.float32)
        r2i = r2_64[win].astype(np.float32)
        jh, jl = _split2(r2j)
        ih, il = _split2(r2i)
        ones_j = np.ones(N, np.float16)
        ones_i = np.ones(COLS, np.float16)
        rows_j += [jh, jl, ones_j, ones_j]
        rows_i += [ones_i, ones_i, ih, il]

        in_maps.append({
            "aj": np.stack(rows_j).astype(np.float16),          # [K, N]
            "bi": np.stack(rows_i).astype(np.float16),          # [K, COLS]
            "qj": q[perm].reshape(NB, P).T.astype(np.float16),  # [P, NB]
            "qi": q[win].reshape(1, COLS).astype(np.float32),   # [1, COLS]
            "zmask": z,                                         # [P, 896]
        })
    return in_maps


def _build_program():
    import concourse.mybir as mybir
    import concourse.tile as tile
    from concourse import bacc

    dt = mybir.dt
    nc = bacc.Bacc("TRN2", target_bir_lowering=False, debug=False,
                   num_devices=NCORES)

    aj = nc.dram_tensor("aj", [K, N], dt.float16, kind="ExternalInput")
    bi = nc.dram_tensor("bi", [K, COLS], dt.float16, kind="ExternalInput")
    qj = nc.dram_tensor("qj", [P, NB], dt.float16, kind="ExternalInput")
    qi = nc.dram_tensor("qi", [1, COLS], dt.float32, kind="ExternalInput")
    zmask = nc.dram_tensor("zmask", [P, 896], dt.float32, kind="ExternalInput")
    pot = nc.dram_tensor("pot", [1, 1], dt.float32, kind="ExternalOutput")

    erf_fn = mybir.ActivationFunctionType.Erf
    sqrt_fn = mybir.ActivationFunctionType.Sqrt
    SW = 6 * COLS   # super-group width (elements in free dim)

    with tile.TileContext(nc) as tc:
        with (
            tc.tile_pool(name="const", bufs=1) as cpool,
            tc.tile_pool(name="work", bufs=3) as wpool,
            tc.tile_pool(name="distk", bufs=len(SUPERS)) as dpool,
            tc.tile_pool(name="rinvk", bufs=len(SUPERS)) as rpool,
            tc.tile_pool(name="d2pool", bufs=1, space="PSUM") as ppool,
            tc.tile_pool(name="spool", bufs=1, space="PSUM") as spool,
        ):
            AJ = cpool.tile([K, N], dt.float16)
            nc.sync.dma_start(AJ[:], aj[:])
            BI = cpool.tile([K, COLS], dt.float16)
            nc.sync.dma_start(BI[:], bi[:])
            QJ = cpool.tile([P, NB], dt.float16)
            nc.gpsimd.dma_start(QJ[:], qj[:])
            QI = cpool.tile([1, COLS], dt.float32)
            nc.gpsimd.dma_start(QI[:], qi[:])
            ZM = cpool.tile([P, 896], dt.float32)
            nc.gpsimd.dma_start(ZM[:], zmask[:])

            d2 = ppool.tile([P, SW], dt.float32)    # 6 PSUM banks, one tile
            s_ps = spool.tile([1, COLS], dt.float32)

            # ---- phase A: d2 matmuls + Sqrt (one ACT table) + recip ----
            dist_tiles, rinv_tiles = [], []
            jb0 = 0
            for g, gsz in enumerate(SUPERS):
                fd = gsz * COLS
                half = fd // 2
                dist = dpool.tile([P, SW], dt.float32, tag="dist")
                rinv = rpool.tile([P, SW], dt.float32, tag="rinv")
                for k in range(gsz):
                    jb = jb0 + k
                    nc.tensor.matmul(
                        d2[:, k * COLS:(k + 1) * COLS],
                        AJ[:, jb * P:(jb + 1) * P],
                        BI[:, :],
                        start=True, stop=True,
                    )
                    # half-super Sqrt: ACT reads finished banks while PE
                    # fills the rest (bank-level PSUM dependency tracking)
                    if gsz > 1 and k == gsz // 2 - 1:
                        nc.scalar.activation(dist[:, :half], d2[:, :half],
                                             sqrt_fn)
                nc.scalar.activation(dist[:, half:fd], d2[:, half:fd], sqrt_fn)
                if gsz == 1:
                    pass
                nc.vector.reciprocal_approx_fast(rinv[:, :fd], dist[:, :fd])
                dist_tiles.append(dist)
                rinv_tiles.append(rinv)
                jb0 += gsz

            # ---- phase B: Erf (single table switch), mask, w, reduction ----
            jb0 = 0
            for g, gsz in enumerate(SUPERS):
                fd = gsz * COLS
                dist = dist_tiles[g]
                rinv = rinv_tiles[g]
                nc.scalar.activation(dist[:, :fd], dist[:, :fd], erf_fn,
                                     scale=float(A))
                for k in range(gsz):
                    jb = jb0 + k
                    if jb >= NB - 4:
                        b = jb - (NB - 4)
                        off = (3 - b) * P
                        nc.vector.tensor_mul(
                            dist[:, k * COLS:(k + 1) * COLS],
                            dist[:, k * COLS:(k + 1) * COLS],
                            ZM[:, off:off + COLS],
                        )
                w = wpool.tile([P, SW], dt.float16, tag="w")
                eng = nc.gpsimd if g in GPSIMD_SUPERS else nc.vector
                eng.tensor_mul(w[:, :fd], dist[:, :fd], rinv[:, :fd])
                for k in range(gsz):
                    jb = jb0 + k
                    nc.tensor.matmul(
                        s_ps[:, :],
                        QJ[:, jb:jb + 1],
                        w[:, k * COLS:(k + 1) * COLS],
                        start=(jb == 0), stop=(jb == NB - 1),
                    )
                jb0 += gsz

            sq = cpool.tile([1, COLS], dt.float32)
            nc.vector.tensor_mul(sq[:, :], s_ps[:, :], QI[:, :])
            acc = cpool.tile([1, 1], dt.float32)
            nc.vector.tensor_reduce(acc[:, :], sq[:, :],
                                    axis=mybir.AxisListType.X,
                                    op=mybir.AluOpType.add)
            pot_sb = cpool.tile([1, 1], dt.float32)
            nc.scalar.mul(pot_sb[:, :], acc[:, :], float(NORM_CONST * 0.5))
            nc.sync.dma_start(pot[:, :], pot_sb[:, :])

    nc.compile()
    return nc


def _get_program():
    if "nc" not in _CACHE:
        _CACHE["nc"] = _build_program()
    return _CACHE["nc"]


def _run(q, r, trace=False, **trace_kwargs):
    from concourse.bass_utils import run_bass_kernel_spmd

    nc = _get_program()
    in_maps = _build_core_inputs(np.asarray(q), np.asarray(r))
    res = run_bass_kernel_spmd(nc, in_maps, core_ids=list(range(NCORES)),
                               trace=trace, **trace_kwargs)
    total = np.float64(0.0)
    for m in res.results:
        total += np.float64(m["pot"].reshape(-1)[0])
    return np.array([total], dtype=np.float32), res


def kernel(q, r, cell=None, batch=None):
    out, _ = _run(q, r, trace=False)
    return out


# revision 8
# speedup vs baseline: 1.3367x; 1.3367x over previous
"""Ewald real-space potential kernel for Trainium2 (8 NeuronCores, SPMD).

Computes pot = 0.5 * sum_{i != j} q_i * erf(d_ij / sqrt(2)) / d_ij * C  over
all pairs of N=4096 atoms, C = 90.0474 / (2*pi).

Sharding: each core owns a 512-column i-slice of the 4096x4096 pair matrix.
On-chip layout per work tile is [128 j-partitions, 512 i-free]; j-blocks are
processed in super-groups of 6 (one [128, 3072] PSUM tensor, PE fills one
3-block half while ACT reads the other via bank-level dependency tracking):
  1. PE     : d2 = |r_i - r_j|^2 via a K=16 fp16 hi/lo-split matmul
              (fp16 products are exact in fp32 PSUM; |err| < 8e-3). The
              core's own j-window carries +0.01 on |r_j|^2 so the masked
              diagonal stays positive under that error; real pairs are
              unaffected (closest pair d2 = 0.0144).
  2. ACT    : dist = Sqrt(d2)            (PSUM -> SBUF, one op per super)
  3. DVE    : rinv = reciprocal_approx_fast(dist)   (~51 ULP)
  4. ACT    : erf_t = Erf(dist / sqrt(2))
  5. DVE    : zero erf_t on the self-interaction diagonal (mask multiply)
  6. DVE/GP : w = erf_t * rinv   (cast fp16, kept resident in SBUF)
  7. PE     : s[1,512] += q_j^T @ w  -- all 32 fp16 matmuls emitted after the
              elementwise loop so the PE FIFO never blocks d2 production.
  8. DVE/ACT: pot = C/2 * sum_i q_i * s_i
The j-axis is rotated per core so each core's own diagonal window lands in
j-blocks 28..31, letting one shared mask tensor serve all cores (SPMD).
Host sums the 8 per-core partial potentials.
"""

import numpy as np

P = 128
N = 4096
NCORES = 8
COLS = N // NCORES          # 512 i-columns per core
NB = N // P                 # 32 j-blocks
K = 16                      # split-matmul contraction rows
SIGMA = 1.0
NORM_CONST = 90.0474 / (2.0 * np.pi)
A = 1.0 / (SIGMA * np.sqrt(2.0))
DIAG_EPS = 0.01             # |r_j|^2 bias on the core's own j-window only
# super-groups: j-blocks per elementwise op; last group covers the remainder
SUPERS = [6, 6, 6, 6, 6, 2]
# which super-groups run the final w-multiply on gpsimd (rest on vector)
GPSIMD_SUPERS = frozenset({0, 3})

_CACHE = {}


def _split2(v32):
    h = v32.astype(np.float16)
    l = (v32 - h.astype(np.float32)).astype(np.float16)
    return h, l


def _build_core_inputs(q, r):
    """Per-core input arrays (keyed by dram tensor name)."""
    q = q.astype(np.float32)
    r = r.astype(np.float32)
    r2_64 = (r.astype(np.float64) ** 2).sum(1)

    # shared diagonal mask: Z[p, u] = 0 iff u == p + 384  (sliced per j-block)
    z = np.ones((P, 896), np.float32)
    z[np.arange(P), np.arange(P) + 384] = 0.0

    in_maps = []
    for c in range(NCORES):
        perm = (np.arange(N) + COLS * (c + 1)) % N  # rotated j order
        win = slice(COLS * c, COLS * (c + 1))       # this core's i-window

        rows_j, rows_i = [], []
        for d in range(3):
            cj = r[perm, d]
            ui = (-2.0 * r[win, d]).astype(np.float32)
            jh, jl = _split2(cj)
            ih, il = _split2(ui)
            rows_j += [jh, jh, jl, jl]
            rows_i += [ih, il, ih, il]
        r2j = r2_64[perm].copy()
        r2j[N - COLS:] += DIAG_EPS      # rotated slots 3584.. = own window
        r2j = r2j.astype(np.float32)
        r2i = r2_64[win].astype(np.float32)
        jh, jl = _split2(r2j)
        ih, il = _split2(r2i)
        ones_j = np.ones(N, np.float16)
        ones_i = np.ones(COLS, np.float16)
        rows_j += [jh, jl, ones_j, ones_j]
        rows_i += [ones_i, ones_i, ih, il]

        in_maps.append({
            "aj": np.stack(rows_j).astype(np.float16),          # [K, N]
            "bi": np.stack(rows_i).astype(np.float16),          # [K, COLS]
            "qj": q[perm].reshape(NB, P).T.astype(np.float16),  # [P, NB]
            "qi": q[win].reshape(1, COLS).astype(np.float32),   # [1, COLS]
            "zmask": z,                                         # [P, 896]
        })
    return in_maps


def _build_program():
    import concourse.mybir as mybir
    import concourse.tile as tile
    from concourse import bacc

    dt = mybir.dt
    nc = bacc.Bacc("TRN2", target_bir_lowering=False, debug=False,
                   num_devices=NCORES)

    aj = nc.dram_tensor("aj", [K, N], dt.float16, kind="ExternalInput")
    bi = nc.dram_tensor("bi", [K, COLS], dt.float16, kind="ExternalInput")
    qj = nc.dram_tensor("qj", [P, NB], dt.float16, kind="ExternalInput")
    qi = nc.dram_tensor("qi", [1, COLS], dt.float32, kind="ExternalInput")
    zmask = nc.dram_tensor("zmask", [P, 896], dt.float32, kind="ExternalInput")
    pot = nc.dram_tensor("pot", [1, 1], dt.float32, kind="ExternalOutput")

    erf_fn = mybir.ActivationFunctionType.Erf
    sqrt_fn = mybir.ActivationFunctionType.Sqrt
    SW = 6 * COLS   # super-group width (elements in free dim)

    with tile.TileContext(nc) as tc:
        with (
            tc.tile_pool(name="const", bufs=1) as cpool,
            tc.tile_pool(name="work", bufs=3) as wpool,
            tc.tile_pool(name="distk", bufs=len(SUPERS)) as dpool,
            tc.tile_pool(name="rinvk", bufs=len(SUPERS)) as rpool,
            tc.tile_pool(name="d2pool", bufs=2, space="PSUM") as ppool,
            tc.tile_pool(name="spool", bufs=1, space="PSUM") as spool,
        ):
            AJ = cpool.tile([K, N], dt.float16)
            nc.sync.dma_start(AJ[:], aj[:])
            BI = cpool.tile([K, COLS], dt.float16)
            nc.sync.dma_start(BI[:], bi[:])
            QJ = cpool.tile([P, NB], dt.float16)
            nc.gpsimd.dma_start(QJ[:], qj[:])
            QI = cpool.tile([1, COLS], dt.float32)
            nc.gpsimd.dma_start(QI[:], qi[:])
            ZM = cpool.tile([P, 896], dt.float32)
            nc.gpsimd.dma_start(ZM[:], zmask[:])

            s_ps = spool.tile([1, COLS], dt.float32)

            # ---- phase A: d2 matmuls + Sqrt (one ACT table) + recip ----
            # PSUM: two [128, 1536] d2 buffers (3 banks each) so PE fills one
            # while ACT drains the other; dist/rinv are wide [128, 3072]
            # tiles written in halves and consumed whole in phase B.
            dist_tiles, rinv_tiles = [], []
            jb0 = 0
            for g, gsz in enumerate(SUPERS):
                fd = gsz * COLS
                dist = dpool.tile([P, SW], dt.float32, tag="dist")
                rinv = rpool.tile([P, SW], dt.float32, tag="rinv")
                nh = max(1, gsz // 3)
                for h in range(nh):
                    hsz = gsz // nh
                    c0 = h * hsz * COLS
                    d2 = ppool.tile([P, 3 * COLS], dt.float32, tag="d2")
                    for k in range(hsz):
                        jb = jb0 + h * hsz + k
                        nc.tensor.matmul(
                            d2[:, k * COLS:(k + 1) * COLS],
                            AJ[:, jb * P:(jb + 1) * P],
                            BI[:, :],
                            start=True, stop=True,
                        )
                    nc.scalar.activation(dist[:, c0:c0 + hsz * COLS],
                                         d2[:, :hsz * COLS], sqrt_fn)
                nc.vector.reciprocal_approx_fast(rinv[:, :fd], dist[:, :fd])
                dist_tiles.append(dist)
                rinv_tiles.append(rinv)
                jb0 += gsz

            # ---- phase B: Erf (single table switch), mask, w, reduction ----
            jb0 = 0
            for g, gsz in enumerate(SUPERS):
                fd = gsz * COLS
                dist = dist_tiles[g]
                rinv = rinv_tiles[g]
                nc.scalar.activation(dist[:, :fd], dist[:, :fd], erf_fn,
                                     scale=float(A))
                for k in range(gsz):
                    jb = jb0 + k
                    if jb >= NB - 4:
                        b = jb - (NB - 4)
                        off = (3 - b) * P
                        nc.vector.tensor_mul(
                            dist[:, k * COLS:(k + 1) * COLS],
                            dist[:, k * COLS:(k + 1) * COLS],
                            ZM[:, off:off + COLS],
                        )
                w = wpool.tile([P, SW], dt.float16, tag="w")
                eng = nc.gpsimd if g in GPSIMD_SUPERS else nc.vector
                eng.tensor_mul(w[:, :fd], dist[:, :fd], rinv[:, :fd])
                for k in range(gsz):
                    jb = jb0 + k
                    nc.tensor.matmul(
                        s_ps[:, :],
                        QJ[:, jb:jb + 1],
                        w[:, k * COLS:(k + 1) * COLS],
                        start=(jb == 0), stop=(jb == NB - 1),
                    )
                jb0 += gsz

            sq = cpool.tile([1, COLS], dt.float32)
            nc.vector.tensor_mul(sq[:, :], s_ps[:, :], QI[:, :])
            acc = cpool.tile([1, 1], dt.float32)
            nc.vector.tensor_reduce(acc[:, :], sq[:, :],
                                    axis=mybir.AxisListType.X,
                                    op=mybir.AluOpType.add)
            pot_sb = cpool.tile([1, 1], dt.float32)
            nc.scalar.mul(pot_sb[:, :], acc[:, :], float(NORM_CONST * 0.5))
            nc.sync.dma_start(pot[:, :], pot_sb[:, :])

    nc.compile()
    return nc


def _get_program():
    if "nc" not in _CACHE:
        _CACHE["nc"] = _build_program()
    return _CACHE["nc"]


def _run(q, r, trace=False, **trace_kwargs):
    from concourse.bass_utils import run_bass_kernel_spmd

    nc = _get_program()
    in_maps = _build_core_inputs(np.asarray(q), np.asarray(r))
    res = run_bass_kernel_spmd(nc, in_maps, core_ids=list(range(NCORES)),
                               trace=trace, **trace_kwargs)
    total = np.float64(0.0)
    for m in res.results:
        total += np.float64(m["pot"].reshape(-1)[0])
    return np.array([total], dtype=np.float32), res


def kernel(q, r, cell=None, batch=None):
    out, _ = _run(q, r, trace=False)
    return out
